# revision 14
# baseline (speedup 1.0000x reference)
"""Chunked (= full, non-causal) multi-head self-attention on 8 TRN2 NeuronCores.

Problem: B=2, S=2048, D=1024, H=16 heads (head_dim 64), torch-Linear-style
projections (y = x @ W.T + b), softmax attention, output projection.

Sharding: head-parallel. Core c owns heads {2c, 2c+1} = feature slice
[128c, 128c+128). Each core computes q/k/v for its slice from the full x
(replicated), runs attention for its 4 (batch, head) pairs, and produces a
partial output projection with its 128-row slice of Wo. Host sums the 8
partials (bf16) and adds bo.

Layout: scores are computed transposed, ST[k, q] (keys on partitions), so the
softmax exp output PT feeds the P@V matmul directly (contraction over k on
partitions) with no on-chip transposes anywhere — x and the weights are
pre-swizzled on the host into [128-partition, chunk, free] form so every DMA
is 128 descriptors of large contiguous runs. The two heads' K=64 score
matmuls land on PE row-groups 0-1/2-3. The softmax denominator rides as row
64 of the PV output via a ones-column appended to V (M=65).

Softmax exp is split across TWO engines: ACT does most tiles (table exp,
1 elem/cycle @1.2GHz) and the otherwise-idle DVE computes a bounded share
via a Schraudolph bit-trick (one tensor_scalar mult+add that writes
round(x*128/ln2 + 16250) as int16 == the bf16 bit pattern of exp(x), ±4.6%).
The denominator sums the same approximated values, so the softmax stays
self-consistent and the end-to-end error stays ~1e-2. y copies (PSUM->SBUF)
are likewise load-balanced between ACT (activation-copy) and DVE.

Scheduling: one software-pipelined stream designed to keep the PE densely
busy (TRN2's HAM clock-gate halves the PE clock after ~3.4us of idle):
dummy warm-up matmuls run during the initial DMAs, only sblock 0's
projections run before attention starts, and all remaining projections +
deferred output projections drip-feed into the attention stream as fillers.
The reciprocal (1/rowsum) is exp(-ln(x)) on ACT — one batched [1,2,512]
Ln+Exp per iteration — broadcast on GPSIMD. The final iteration normalizes
straight out of PSUM while dummy matmuls keep the PE clock warm for the
tail output projections.
"""

import sys

if "/opt/trn_rl_repo" not in sys.path:
    sys.path.insert(0, "/opt/trn_rl_repo")

import numpy as np

import concourse.bacc as bacc
import concourse.mybir as mybir
import concourse.tile as tile
from concourse import bass_utils

# Route Exp to the activation-table set that also holds Ln (and Copy), so the
# softmax exps, the reciprocal-via-exp(-ln(x)) trick and the ACT y-copies all
# share one table set (a set switch costs ~2.7us).
_orig_get_activation_tables = bacc.get_activation_tables


def _patched_get_activation_tables(arch):
    out = {}
    for name, funcs in dict(_orig_get_activation_tables(arch)).items():
        if name != "natural_log_exp_and_others":
            funcs = {f for f in funcs if f != mybir.ActivationFunctionType.Exp}
        out[name] = funcs
    return out


bacc.get_activation_tables = _patched_get_activation_tables

B, S, D, H = 2, 2048, 1024, 16
HD = D // H          # 64
NCORES = 8
ES = D // NCORES     # 128 features (= 2 heads) per core
BS = B * S           # 4096 rows total

P = 128              # partitions
NF = 512             # matmul free-dim tile
N_SB = BS // NF      # 8 s-blocks of 512
N_DC = D // P        # 8 contraction chunks of 128
N_KB = S // P        # 16 key blocks of 128 per batch
N_KP = N_KB // 2     # 8 key-block PAIRS per batch
N_QC = S // NF       # 4 query chunks of 512 per batch
N_CH = BS // P       # 32 global 128-row chunks

F32 = mybir.dt.float32
BF16 = mybir.dt.bfloat16
I16 = mybir.dt.int16

DT = BF16            # activations / weights on the PE

# Schraudolph exp on DVE: int16 bits = round(score * ALPHA + BETA) viewed as
# bf16 ~= exp(score / sqrt(HD)). Round-to-nearest-even cast verified on HW.
INV_SQRT_HD = 1.0 / float(np.sqrt(HD))
SCH_ALPHA = INV_SQRT_HD * 1.4426950408889634 * 128.0
SCH_BETA = 16256.0 - 6.0
DVE_EXP_CAP = 20     # max score tiles exp'd on DVE (accuracy-bounded)

N_WARM = 16          # PE warm-up dummy matmuls during initial DMA
N_TAIL_WARM = 16     # PE keep-warm dummies during the final normalize chain

# engine-load estimates (ns) for the greedy ACT/DVE balancer
C_ACT_EXP, C_DVE_EXP = 1330.0, 1270.0
C_ACT_Y, C_DVE_Y = 640.0, 700.0
C_ACT_RECIP = 2100.0
C_ACT_QK, C_DVE_QK = 610.0, 760.0
C_DVE_V, C_DVE_ORAW, C_DVE_APPLY = 340.0, 1270.0, 1370.0

_cache = {}
last_results = None          # test.py reads exec_time_ns off this


def _np_dt(dt):
    import ml_dtypes

    return np.dtype(ml_dtypes.bfloat16) if dt == mybir.dt.bfloat16 else np.dtype(np.float32)


def _build():
    nc = bacc.Bacc("TRN2", target_bir_lowering=False, debug=False)

    # host pre-swizzled inputs: [128-partition, chunk, free] everywhere
    xT_d = nc.dram_tensor("xT", [P, N_SB, N_DC, NF], DT, kind="ExternalInput")
    wqT_d = nc.dram_tensor("wqT", [P, N_DC, ES], DT, kind="ExternalInput")
    wkT_d = nc.dram_tensor("wkT", [P, N_DC, ES], DT, kind="ExternalInput")
    wvT_d = nc.dram_tensor("wvT", [P, N_DC, ES], DT, kind="ExternalInput")
    bqk_d = nc.dram_tensor("bqk", [ES, 2], F32, kind="ExternalInput")
    bv_d = nc.dram_tensor("bv", [1, ES], F32, kind="ExternalInput")
    woT_d = nc.dram_tensor("woT", [ES, D], DT, kind="ExternalInput")
    y_d = nc.dram_tensor("y", [BS, D], BF16, kind="ExternalOutput")

    with tile.TileContext(nc) as tc:
        with tc.tile_pool(name="const", bufs=1) as cpool, \
             tc.tile_pool(name="xt", bufs=3) as xt_pool, \
             tc.tile_pool(name="qkv", bufs=1) as qkv_pool, \
             tc.tile_pool(name="pt", bufs=14) as pt_pool, \
             tc.tile_pool(name="ysb", bufs=4) as y_pool, \
             tc.tile_pool(name="ps", bufs=1, space="PSUM") as ps:

            # ---- engine load balancer --------------------------------
            load = {"act": 0.0, "dve": 0.0}
            dve_exp_used = [0]

            def pick(c_act, c_dve, dve_ok=True):
                if dve_ok and load["dve"] + c_dve < load["act"] + c_act:
                    load["dve"] += c_dve
                    return "dve"
                load["act"] += c_act
                return "act"

            # ---- input DMAs, priority order --------------------------
            strips = {}

            def emit_strip_dma(sb):
                strip = xt_pool.tile([P, N_DC, NF], DT, tag="strip",
                                     name=f"strip{sb}")
                nc.sync.dma_start(strip[:], xT_d[:, sb])
                strips[sb] = strip

            wq_sb = cpool.tile([P, N_DC, ES], DT)
            wk_sb = cpool.tile([P, N_DC, ES], DT)
            wv_sb = cpool.tile([P, N_DC, ES], DT)
            bqk_sb = cpool.tile([ES, 2], F32)
            bv_row = cpool.tile([1, ES], F32)
            wo_sb = cpool.tile([ES, D], DT)

            emit_strip_dma(0)
            nc.sync.dma_start(wk_sb[:], wkT_d[:])
            nc.sync.dma_start(wq_sb[:], wqT_d[:])
            nc.sync.dma_start(wv_sb[:], wvT_d[:])
            emit_strip_dma(1)
            emit_strip_dma(2)
            # small/late-needed transfers ride the idle GPSIMD queue
            nc.gpsimd.dma_start(bqk_sb[:], bqk_d[:])
            nc.gpsimd.dma_start(bv_row[:], bv_d[:])
            nc.gpsimd.dma_start(wo_sb[:], woT_d[:])

            # ---- PE warm-up during the DMAs --------------------------
            warm = cpool.tile([P, NF], DT)
            nc.vector.memset(warm[:], 0.0)

            def emit_dummy_mm():
                wm = ps.tile([P, NF], F32, tag="misc", bufs=2)
                nc.tensor.matmul(wm[:], warm[:, 0:P], warm[:],
                                 start=True, stop=True)

            for _ in range(N_WARM):
                emit_dummy_mm()

            # ---- constants -------------------------------------------
            ones_row = cpool.tile([1, ES], F32)
            nc.vector.memset(ones_row[:], 1.0)
            # bv broadcast to all 128 partitions via rank-1 matmul
            bv_bc_ps = ps.tile([P, ES], F32, tag="misc", bufs=2)
            nc.tensor.matmul(bv_bc_ps[:], ones_row[:], bv_row[:],
                             start=True, stop=True)
            bv_bc = cpool.tile([P, 2, HD], F32)
            nc.vector.tensor_copy(bv_bc[:], bv_bc_ps[:].rearrange(
                "p (a b) -> p a b", a=2))

            # ---- persistent activations ------------------------------
            qT_sb = qkv_pool.tile([P, BS], DT)     # [feat 128, s 4096]
            kT_sb = qkv_pool.tile([P, BS], DT)
            # V for both heads + ones cols: [k-part, chunk, head, HD+1]
            vAB_sb = qkv_pool.tile([P, N_CH, 2, HD + 1], DT)
            oT_sb = qkv_pool.tile([P, BS], DT)     # normalized attn out
            nc.vector.memset(vAB_sb[:, :, :, HD : HD + 1], 1.0)

            # ---- emission helpers ------------------------------------
            def emit_qk_piece(sb, which):
                s0 = sb * NF
                strip = strips[sb]
                w_sb, bidx, dst = ((wq_sb, 0, qT_sb) if which == "q"
                                   else (wk_sb, 1, kT_sb))
                p_ps = ps.tile([P, NF], F32, tag="misc", bufs=2,
                               name=f"{which}{sb}_ps")
                for j in range(N_DC):
                    nc.tensor.matmul(p_ps[:], w_sb[:, j], strip[:, j],
                                     start=(j == 0), stop=(j == N_DC - 1))
                if pick(C_ACT_QK, C_DVE_QK) == "act":
                    nc.scalar.activation(dst[:, s0 : s0 + NF], p_ps[:],
                                         mybir.ActivationFunctionType.Identity,
                                         bias=bqk_sb[:, bidx : bidx + 1],
                                         scale=1.0)
                else:
                    nc.vector.tensor_scalar_add(dst[:, s0 : s0 + NF], p_ps[:],
                                                bqk_sb[:, bidx : bidx + 1])

            def emit_v_piece(sb, ss):
                strip = strips[sb]
                ch = sb * (NF // P) + ss
                v_ps = ps.tile([P, 2, HD], F32, tag="misc", bufs=2,
                               name=f"v{ch}_ps")
                v_flat = v_ps[:].rearrange("p a b -> p (a b)")
                for j in range(N_DC):
                    nc.tensor.matmul(v_flat, strip[:, j, ss * P : (ss + 1) * P],
                                     wv_sb[:, j],
                                     start=(j == 0), stop=(j == N_DC - 1))
                nc.vector.tensor_add(vAB_sb[:, ch, :, 0:HD], v_ps[:], bv_bc[:])
                load["dve"] += C_DVE_V

            # ---- prologue: sblock 0 ----------------------------------
            emit_qk_piece(0, "k")
            emit_qk_piece(0, "q")
            for ss in range(NF // P):
                emit_v_piece(0, ss)

            # filler queue: remaining b0 work (k-first, v close behind),
            # then all b1 projections. Drip-fed into the attention stream.
            # NOTE: pop pacing is exactly-in-time against ST/PV emission
            # order — pieces must be EMITTED before their consumer.
            a_queue = [("k", 1), ("v", 1, 0), ("v", 1, 1), ("v", 1, 2),
                       ("v", 1, 3), ("k", 2), ("dma", 3), ("k", 3), ("q", 1),
                       ("v", 2, 0), ("v", 2, 1), ("v", 2, 2), ("v", 2, 3),
                       ("q", 2),
                       ("v", 3, 0), ("v", 3, 1), ("v", 3, 2), ("v", 3, 3),
                       ("q", 3)]
            for sb in range(N_SB // 2, N_SB):
                a_queue.append(("dma", sb))
                a_queue.append(("k", sb))
                a_queue.append(("q", sb))
                for ss in range(NF // P):
                    a_queue.append(("v", sb, ss))

            def emit_a_piece():
                piece = a_queue.pop(0)
                if piece[0] == "dma":
                    emit_strip_dma(piece[1])
                    if a_queue:
                        emit_a_piece()  # dma is async; also emit compute
                elif piece[0] in ("q", "k"):
                    emit_qk_piece(piece[1], piece[0])
                else:
                    emit_v_piece(piece[1], piece[2])

            y_queue = []

            def emit_y_unit(s0, force_split=False):
                # one 128-row block of y, all 1024 output features
                y2 = y_pool.tile([P, 2, NF], BF16, tag="y")
                for ec in range(2):
                    y_ps = ps.tile([P, NF], F32, tag="misc", bufs=2)
                    nc.tensor.matmul(y_ps[:], oT_sb[:, s0 : s0 + P],
                                     wo_sb[:, ec * NF : (ec + 1) * NF],
                                     start=True, stop=True)
                    if force_split:
                        eng = "act" if ec == 0 else "dve"
                    else:
                        eng = pick(C_ACT_Y, C_DVE_Y)
                    if eng == "act":
                        nc.scalar.copy(y2[:, ec], y_ps[:])
                    else:
                        nc.vector.tensor_copy(y2[:, ec], y_ps[:])
                nc.sync.dma_start(y_d[s0 : s0 + P, :],
                                  y2[:].rearrange("p a b -> p (a b)"))

            def emit_recip_chain(o_src, q0):
                # 1/rowsum as exp(-ln(x)) on ACT, one batched call per iter;
                # partition-broadcast on the idle GPSIMD
                lg2 = pt_pool.tile([1, 2, NF], F32, tag="lg", bufs=4)
                rcp2 = pt_pool.tile([1, 2, NF], F32, tag="rcp", bufs=4)
                nc.scalar.activation(lg2[:], o_src[HD : HD + 1],
                                     mybir.ActivationFunctionType.Ln)
                nc.scalar.activation(rcp2[:], lg2[:],
                                     mybir.ActivationFunctionType.Exp,
                                     scale=-1.0)
                load["act"] += C_ACT_RECIP
                bc2 = pt_pool.tile([HD, 2, NF], F32, tag="bc", bufs=3)
                nc.gpsimd.partition_broadcast(bc2[:], rcp2[:])
                return (o_src, bc2, q0)

            def emit_apply(o_src, bc2, q0):
                for hidx, part in ((0, 0), (1, HD)):
                    nc.vector.tensor_mul(
                        oT_sb[part : part + HD, q0 : q0 + NF],
                        o_src[0:HD, hidx], bc2[:, hidx])
                load["dve"] += C_DVE_APPLY
                for ss in range(NF // P):
                    y_queue.append(q0 + ss * P)

            # ---- attention: one continuous software pipeline ---------
            n_iters = B * N_QC
            total_pairs = n_iters * N_KP
            o_tiles = {}
            ptq = {}
            pending = None
            norm_state = None

            for g in range(total_pairs + 1):
                if g < total_pairs:
                    it = g // N_KP
                    kp = g % N_KP
                    b, qc = it // N_QC, it % N_QC
                    if kp == 0 and b == 1 and qc == 0:
                        while a_queue:
                            emit_a_piece()
                    q0 = b * S + qc * NF
                    st2A = ps.tile([P, 2, NF], F32, tag="st2", bufs=2)
                    st2B = ps.tile([P, 2, NF], F32, tag="st2", bufs=2)
                    for half in range(2):
                        k0 = b * S + (kp * 2 + half) * P
                        nc.tensor.matmul(st2A[:, half], kT_sb[0:HD, k0 : k0 + P],
                                         qT_sb[0:HD, q0 : q0 + NF],
                                         start=True, stop=True)
                        nc.tensor.matmul(st2B[:, half], kT_sb[HD:P, k0 : k0 + P],
                                         qT_sb[HD:P, q0 : q0 + NF],
                                         start=True, stop=True)
                    pts = []
                    for st2 in (st2A, st2B):
                        eng = pick(C_ACT_EXP, C_DVE_EXP,
                                   dve_ok=dve_exp_used[0] < DVE_EXP_CAP)
                        pt2 = pt_pool.tile([P, 2, NF], DT, tag="pt", bufs=14)
                        if eng == "act":
                            nc.scalar.activation(
                                pt2[:], st2[:],
                                mybir.ActivationFunctionType.Exp,
                                scale=INV_SQRT_HD)
                        else:
                            dve_exp_used[0] += 1
                            nc.vector.tensor_scalar(
                                out=pt2[:].bitcast(I16), in0=st2[:],
                                scalar1=SCH_ALPHA, scalar2=SCH_BETA,
                                op0=mybir.AluOpType.mult,
                                op1=mybir.AluOpType.add)
                        pts.append(pt2)
                    ptq[g] = tuple(pts)

                    # fillers ride the ST side of the stream
                    if a_queue:
                        emit_a_piece()
                        if b == 0 and a_queue:
                            emit_a_piece()
                    elif y_queue:
                        emit_y_unit(y_queue.pop(0))
                    if kp == 1 and pending is not None:
                        norm_state = emit_recip_chain(*pending)
                        pending = None
                    if kp == 4 and norm_state is not None:
                        emit_apply(*norm_state)
                        norm_state = None

                if g >= 1:
                    pg = g - 1
                    it = pg // N_KP
                    kp = pg % N_KP
                    b, qc = it // N_QC, it % N_QC
                    q0 = b * S + qc * NF
                    if kp == 0:
                        o2 = ps.tile([HD + 1, 2, NF], F32, tag="o", bufs=1)
                        o_tiles[it] = o2
                    o2 = o_tiles[it]
                    pt2A, pt2B = ptq.pop(pg)
                    for half in range(2):
                        kb = kp * 2 + half
                        gkb = b * N_KB + kb
                        nc.tensor.matmul(o2[:, 0], vAB_sb[:, gkb, 0],
                                         pt2A[:, half],
                                         start=(kb == 0), stop=(kb == N_KB - 1))
                        nc.tensor.matmul(o2[:, 1], vAB_sb[:, gkb, 1],
                                         pt2B[:, half],
                                         start=(kb == 0), stop=(kb == N_KB - 1))
                    if kp == N_KP - 1:
                        del o_tiles[it]
                        if it < n_iters - 1:
                            # free the o banks: copy out, defer normalize
                            o_raw = pt_pool.tile([HD + 1, 2, NF], F32,
                                                 tag="oraw", bufs=4)
                            nc.vector.tensor_copy(o_raw[:], o2[:])
                            load["dve"] += C_DVE_ORAW
                            pending = (o_raw, q0)
                        else:
                            # last iteration: normalize straight from PSUM;
                            # leftover y work + dummies keep the PE warm
                            # through the normalize chain
                            while y_queue:
                                emit_y_unit(y_queue.pop(0), force_split=True)
                            for _ in range(N_TAIL_WARM):
                                emit_dummy_mm()
                            pending = (o2, q0)

            # final normalize, split per 128-query chunk so each tail
            # y-unit (and its DMA) launches as early as possible
            o_src, bc2t, q0t = emit_recip_chain(*pending)
            for ss in range(NF // P):
                qs = slice(ss * P, (ss + 1) * P)
                for hidx, part in ((0, 0), (1, HD)):
                    nc.vector.tensor_mul(
                        oT_sb[part : part + HD, q0t + ss * P : q0t + (ss + 1) * P],
                        o_src[0:HD, hidx, qs], bc2t[:, hidx, qs])
                emit_y_unit(q0t + ss * P, force_split=True)

    nc.compile()
    return nc


def kernel(x, Wq, bq, Wk, bk, Wv, bv, Wo, bo, _trace=False):
    global last_results
    x = np.asarray(x, dtype=np.float32)
    Wq, bq = np.asarray(Wq, np.float32), np.asarray(bq, np.float32)
    Wk, bk = np.asarray(Wk, np.float32), np.asarray(bk, np.float32)
    Wv, bv = np.asarray(Wv, np.float32), np.asarray(bv, np.float32)
    Wo, bo = np.asarray(Wo, np.float32), np.asarray(bo, np.float32)

    if "nc" not in _cache:
        _cache["nc"] = _build()
    nc = _cache["nc"]

    dt = _np_dt(DT)
    # xT strips pre-swizzled to [P, N_SB, N_DC, NF] so each strip DMA is
    # 128 descriptors of contiguous 8KB
    x2 = x.reshape(BS, D)
    xT_pre = np.ascontiguousarray(
        x2.reshape(N_SB, NF, N_DC, P).transpose(3, 0, 2, 1)).astype(dt)

    def wpre(W, sl):
        wT = np.ascontiguousarray(W[sl].T)           # [D, ES]
        return np.ascontiguousarray(
            wT.reshape(N_DC, P, ES).transpose(1, 0, 2)).astype(dt)

    in_maps = []
    for c in range(NCORES):
        sl = slice(c * ES, (c + 1) * ES)
        in_maps.append({
            "xT": xT_pre,
            "wqT": wpre(Wq, sl),
            "wkT": wpre(Wk, sl),
            "wvT": wpre(Wv, sl),
            "bqk": np.ascontiguousarray(
                np.stack([bq[sl], bk[sl]], axis=1).astype(np.float32)),
            "bv": np.ascontiguousarray(bv[None, sl]),
            "woT": np.ascontiguousarray(Wo[:, sl].T).astype(dt),
        })

    res = bass_utils.run_bass_kernel_spmd(
        nc, in_maps, core_ids=list(range(NCORES)), trace=_trace)
    last_results = res

    y = res.results[0]["y"].astype(np.float64)
    for c in range(1, NCORES):
        y += res.results[c]["y"]
    y = (y + bo).astype(np.float32)
    return y.reshape(B, S, D)


# revision 15
# speedup vs baseline: 1.0620x; 1.0620x over previous
"""Chunked (= full, non-causal) multi-head self-attention on 8 TRN2 NeuronCores.

Problem: B=2, S=2048, D=1024, H=16 heads (head_dim 64), torch-Linear-style
projections (y = x @ W.T + b), softmax attention, output projection.

Sharding: head-parallel. Core c owns heads {2c, 2c+1} = feature slice
[128c, 128c+128). Each core computes q/k/v for its slice from the full x
(replicated), runs attention for its 4 (batch, head) pairs, and produces a
partial output projection with its 128-row slice of Wo. Host sums the 8
partials (bf16) and adds bo.

Layout: scores are computed transposed, ST[k, q] (keys on partitions), so the
softmax exp output PT feeds the P@V matmul directly (contraction over k on
partitions) with no on-chip transposes anywhere — x and the weights are
pre-swizzled on the host into [128-partition, chunk, free] form so every DMA
is 128 descriptors of large contiguous runs. The two heads' K=64 score
matmuls land on PE row-groups 0-1/2-3. The softmax denominator rides as row
64 of the PV output via a ones-column appended to V (M=65).

Softmax exp is split across TWO engines: ACT does most tiles (table exp,
1 elem/cycle @1.2GHz) and the otherwise-idle DVE computes a bounded share
via a Schraudolph bit-trick (one tensor_scalar mult+add that writes
round(x*128/ln2 + 16250) as int16 == the bf16 bit pattern of exp(x), ±4.6%).
The denominator sums the same approximated values, so the softmax stays
self-consistent and the end-to-end error stays ~1e-2. y copies (PSUM->SBUF)
are likewise load-balanced between ACT (activation-copy) and DVE.

Scheduling: one software-pipelined stream designed to keep the PE densely
busy (TRN2's HAM clock-gate halves the PE clock after ~3.4us of idle):
dummy warm-up matmuls run during the initial DMAs, only sblock 0's
projections run before attention starts, and all remaining projections +
deferred output projections drip-feed into the attention stream as fillers.
The reciprocal (1/rowsum) is exp(-ln(x)) on ACT — one batched [1,2,512]
Ln+Exp per iteration — broadcast on GPSIMD. The final iteration normalizes
straight out of PSUM while dummy matmuls keep the PE clock warm for the
tail output projections.
"""

import sys

if "/opt/trn_rl_repo" not in sys.path:
    sys.path.insert(0, "/opt/trn_rl_repo")

import numpy as np

import concourse.bacc as bacc
import concourse.mybir as mybir
import concourse.tile as tile
from concourse import bass_utils

# Route Exp to the activation-table set that also holds Ln (and Copy), so the
# softmax exps, the reciprocal-via-exp(-ln(x)) trick and the ACT y-copies all
# share one table set (a set switch costs ~2.7us).
_orig_get_activation_tables = bacc.get_activation_tables


def _patched_get_activation_tables(arch):
    out = {}
    for name, funcs in dict(_orig_get_activation_tables(arch)).items():
        if name != "natural_log_exp_and_others":
            funcs = {f for f in funcs if f != mybir.ActivationFunctionType.Exp}
        out[name] = funcs
    return out


bacc.get_activation_tables = _patched_get_activation_tables

B, S, D, H = 2, 2048, 1024, 16
HD = D // H          # 64
NCORES = 8
ES = D // NCORES     # 128 features (= 2 heads) per core
BS = B * S           # 4096 rows total

P = 128              # partitions
NF = 512             # matmul free-dim tile
N_SB = BS // NF      # 8 s-blocks of 512
N_DC = D // P        # 8 contraction chunks of 128
N_KB = S // P        # 16 key blocks of 128 per batch
N_KP = N_KB // 2     # 8 key-block PAIRS per batch
N_QC = S // NF       # 4 query chunks of 512 per batch
N_CH = BS // P       # 32 global 128-row chunks

F32 = mybir.dt.float32
BF16 = mybir.dt.bfloat16
I16 = mybir.dt.int16

DT = BF16            # activations / weights on the PE

# Schraudolph exp on DVE: int16 bits = round(score * ALPHA + BETA) viewed as
# bf16 ~= exp(score / sqrt(HD)). Round-to-nearest-even cast verified on HW.
INV_SQRT_HD = 1.0 / float(np.sqrt(HD))
SCH_ALPHA = INV_SQRT_HD * 1.4426950408889634 * 128.0
SCH_BETA = 16256.0 - 6.0
DVE_EXP_CAP = 20     # max score tiles exp'd on DVE (accuracy-bounded)

N_WARM = 16          # PE warm-up dummy matmuls during initial DMA
N_TAIL_WARM = 16     # PE keep-warm dummies during the final normalize chain

# engine-load estimates (ns) for the greedy ACT/DVE balancer
C_ACT_EXP, C_DVE_EXP = 1330.0, 1270.0
C_ACT_Y, C_DVE_Y = 640.0, 700.0
C_ACT_RECIP = 2100.0
C_ACT_QK, C_DVE_QK = 610.0, 760.0
C_DVE_V, C_DVE_ORAW, C_DVE_APPLY = 340.0, 1270.0, 1370.0

_cache = {}
last_results = None          # test.py reads exec_time_ns off this


def _np_dt(dt):
    import ml_dtypes

    return np.dtype(ml_dtypes.bfloat16) if dt == mybir.dt.bfloat16 else np.dtype(np.float32)


def _build():
    nc = bacc.Bacc("TRN2", target_bir_lowering=False, debug=False)

    # host pre-swizzled inputs: [128-partition, chunk, free] everywhere
    xT_d = nc.dram_tensor("xT", [P, N_SB, N_DC, NF], DT, kind="ExternalInput")
    wqT_d = nc.dram_tensor("wqT", [P, N_DC, ES], DT, kind="ExternalInput")
    wkT_d = nc.dram_tensor("wkT", [P, N_DC, ES], DT, kind="ExternalInput")
    wvT_d = nc.dram_tensor("wvT", [P, N_DC, ES], DT, kind="ExternalInput")
    bqk_d = nc.dram_tensor("bqk", [ES, 2], F32, kind="ExternalInput")
    bv_d = nc.dram_tensor("bv", [1, ES], F32, kind="ExternalInput")
    woT_d = nc.dram_tensor("woT", [ES, D], DT, kind="ExternalInput")
    y_d = nc.dram_tensor("y", [BS, D], BF16, kind="ExternalOutput")

    with tile.TileContext(nc) as tc:
        with tc.tile_pool(name="const", bufs=1) as cpool, \
             tc.tile_pool(name="xt", bufs=3) as xt_pool, \
             tc.tile_pool(name="qkv", bufs=1) as qkv_pool, \
             tc.tile_pool(name="pt", bufs=14) as pt_pool, \
             tc.tile_pool(name="ysb", bufs=4) as y_pool, \
             tc.tile_pool(name="ps", bufs=1, space="PSUM") as ps:

            # ---- engine load balancer --------------------------------
            load = {"act": 0.0, "dve": 0.0}
            dve_exp_used = [0]

            def pick(c_act, c_dve, dve_ok=True):
                if dve_ok and load["dve"] + c_dve < load["act"] + c_act:
                    load["dve"] += c_dve
                    return "dve"
                load["act"] += c_act
                return "act"

            # ---- input DMAs, priority order --------------------------
            strips = {}

            def emit_strip_dma(sb):
                strip = xt_pool.tile([P, N_DC, NF], DT, tag="strip",
                                     name=f"strip{sb}")
                nc.sync.dma_start(strip[:], xT_d[:, sb])
                strips[sb] = strip

            wq_sb = cpool.tile([P, N_DC, ES], DT)
            wk_sb = cpool.tile([P, N_DC, ES], DT)
            wv_sb = cpool.tile([P, N_DC, ES], DT)
            bqk_sb = cpool.tile([ES, 2], F32)
            bv_row = cpool.tile([1, ES], F32)
            wo_sb = cpool.tile([ES, D], DT)

            emit_strip_dma(0)
            nc.sync.dma_start(wk_sb[:], wkT_d[:])
            nc.sync.dma_start(wq_sb[:], wqT_d[:])
            nc.sync.dma_start(wv_sb[:], wvT_d[:])
            emit_strip_dma(1)
            emit_strip_dma(2)
            # small/late-needed transfers ride the idle GPSIMD queue
            nc.gpsimd.dma_start(bqk_sb[:], bqk_d[:])
            nc.gpsimd.dma_start(bv_row[:], bv_d[:])
            nc.gpsimd.dma_start(wo_sb[:], woT_d[:])

            # ---- PE warm-up during the DMAs --------------------------
            warm = cpool.tile([P, NF], DT)
            nc.vector.memset(warm[:], 0.0)

            def emit_dummy_mm():
                wm = ps.tile([P, NF], F32, tag="misc", bufs=2)
                nc.tensor.matmul(wm[:], warm[:, 0:P], warm[:],
                                 start=True, stop=True)

            for _ in range(N_WARM):
                emit_dummy_mm()

            # ---- constants -------------------------------------------
            ones_row = cpool.tile([1, ES], F32)
            nc.vector.memset(ones_row[:], 1.0)
            # bv broadcast to all 128 partitions via rank-1 matmul
            bv_bc_ps = ps.tile([P, ES], F32, tag="misc", bufs=2)
            nc.tensor.matmul(bv_bc_ps[:], ones_row[:], bv_row[:],
                             start=True, stop=True)
            bv_bc = cpool.tile([P, 2, HD], F32)
            nc.vector.tensor_copy(bv_bc[:], bv_bc_ps[:].rearrange(
                "p (a b) -> p a b", a=2))

            # ---- persistent activations ------------------------------
            qT_sb = qkv_pool.tile([P, BS], DT)     # [feat 128, s 4096]
            kT_sb = qkv_pool.tile([P, BS], DT)
            # V for both heads + ones cols: [k-part, chunk, head, HD+1]
            vAB_sb = qkv_pool.tile([P, N_CH, 2, HD + 1], DT)
            oT_sb = qkv_pool.tile([P, BS], DT)     # normalized attn out
            nc.vector.memset(vAB_sb[:, :, :, HD : HD + 1], 1.0)

            # ---- emission helpers ------------------------------------
            def emit_qk_piece(sb, which):
                s0 = sb * NF
                strip = strips[sb]
                w_sb, bidx, dst = ((wq_sb, 0, qT_sb) if which == "q"
                                   else (wk_sb, 1, kT_sb))
                p_ps = ps.tile([P, NF], F32, tag="misc", bufs=2,
                               name=f"{which}{sb}_ps")
                for j in range(N_DC):
                    nc.tensor.matmul(p_ps[:], w_sb[:, j], strip[:, j],
                                     start=(j == 0), stop=(j == N_DC - 1))
                if pick(C_ACT_QK, C_DVE_QK) == "act":
                    nc.scalar.activation(dst[:, s0 : s0 + NF], p_ps[:],
                                         mybir.ActivationFunctionType.Identity,
                                         bias=bqk_sb[:, bidx : bidx + 1],
                                         scale=1.0)
                else:
                    nc.vector.tensor_scalar_add(dst[:, s0 : s0 + NF], p_ps[:],
                                                bqk_sb[:, bidx : bidx + 1])

            def emit_v_piece(sb, ss):
                strip = strips[sb]
                ch = sb * (NF // P) + ss
                v_ps = ps.tile([P, 2, HD], F32, tag="misc", bufs=2,
                               name=f"v{ch}_ps")
                v_flat = v_ps[:].rearrange("p a b -> p (a b)")
                for j in range(N_DC):
                    nc.tensor.matmul(v_flat, strip[:, j, ss * P : (ss + 1) * P],
                                     wv_sb[:, j],
                                     start=(j == 0), stop=(j == N_DC - 1))
                nc.vector.tensor_add(vAB_sb[:, ch, :, 0:HD], v_ps[:], bv_bc[:])
                load["dve"] += C_DVE_V

            # ---- prologue: k0/q0 only --------------------------------
            emit_qk_piece(0, "k")
            emit_qk_piece(0, "q")

            # Deadline-scheduled fillers. dl = latest pass whose filler
            # slot may EMIT the piece (program order: a pass emits
            # [ST(g), exps, fillers] then [PV(g-1)], so a piece consumed
            # by ST(p) needs dl <= p-1 and by PV(pg) needs dl <= pg+1).
            # The dl-sorted front-only pops also guarantee strip-buffer
            # reuse safety (all readers of strip sb-3 sort before
            # dma(sb): checked for every sb).
            def dl_k(sb):
                return 2 * sb - 1 if sb < 4 else 2 * sb + 23

            def dl_q(sb):
                return 8 * sb - 1 if sb < 4 else 8 * (sb - 4) + 31

            def dl_v(sb, ss):
                ch = 4 * sb + ss
                return ch // 2 + 1 if ch < 16 else (ch - 16) // 2 + 33

            sched = []
            for ss in range(NF // P):
                sched.append((dl_v(0, ss), ("v", 0, ss)))
            for sb in range(1, N_SB):
                if sb >= 3:
                    sched.append((dl_k(sb) - 2, ("dma", sb)))
                sched.append((dl_k(sb), ("k", sb)))
                sched.append((dl_q(sb), ("q", sb)))
                for ss in range(NF // P):
                    sched.append((dl_v(sb, ss), ("v", sb, ss)))
            sched.sort(key=lambda e: e[0])

            def emit_piece(piece):
                if piece[0] == "dma":
                    emit_strip_dma(piece[1])
                elif piece[0] in ("q", "k"):
                    emit_qk_piece(piece[1], piece[0])
                else:
                    emit_v_piece(piece[1], piece[2])

            y_queue = []

            def emit_y_unit(s0, force_split=False):
                # one 128-row block of y, all 1024 output features
                y2 = y_pool.tile([P, 2, NF], BF16, tag="y")
                for ec in range(2):
                    y_ps = ps.tile([P, NF], F32, tag="misc", bufs=2)
                    nc.tensor.matmul(y_ps[:], oT_sb[:, s0 : s0 + P],
                                     wo_sb[:, ec * NF : (ec + 1) * NF],
                                     start=True, stop=True)
                    if force_split:
                        eng = "act" if ec == 0 else "dve"
                    else:
                        eng = pick(C_ACT_Y, C_DVE_Y)
                    if eng == "act":
                        nc.scalar.copy(y2[:, ec], y_ps[:])
                    else:
                        nc.vector.tensor_copy(y2[:, ec], y_ps[:])
                nc.sync.dma_start(y_d[s0 : s0 + P, :],
                                  y2[:].rearrange("p a b -> p (a b)"))

            def emit_recip_chain(o_src, q0):
                # 1/rowsum as exp(-ln(x)) on ACT, one batched call per iter;
                # partition-broadcast on the idle GPSIMD
                lg2 = pt_pool.tile([1, 2, NF], F32, tag="lg", bufs=4)
                rcp2 = pt_pool.tile([1, 2, NF], F32, tag="rcp", bufs=4)
                nc.scalar.activation(lg2[:], o_src[HD : HD + 1],
                                     mybir.ActivationFunctionType.Ln)
                nc.scalar.activation(rcp2[:], lg2[:],
                                     mybir.ActivationFunctionType.Exp,
                                     scale=-1.0)
                load["act"] += C_ACT_RECIP
                bc2 = pt_pool.tile([HD, 2, NF], F32, tag="bc", bufs=3)
                nc.gpsimd.partition_broadcast(bc2[:], rcp2[:])
                return (o_src, bc2, q0)

            def emit_apply(o_src, bc2, q0):
                for hidx, part in ((0, 0), (1, HD)):
                    nc.vector.tensor_mul(
                        oT_sb[part : part + HD, q0 : q0 + NF],
                        o_src[0:HD, hidx], bc2[:, hidx])
                load["dve"] += C_DVE_APPLY
                for ss in range(NF // P):
                    y_queue.append(q0 + ss * P)

            # ---- attention: one continuous software pipeline ---------
            n_iters = B * N_QC
            total_pairs = n_iters * N_KP
            o_tiles = {}
            ptq = {}
            pending = None
            norm_state = None

            for g in range(total_pairs + 1):
                if g < total_pairs:
                    it = g // N_KP
                    kp = g % N_KP
                    b, qc = it // N_QC, it % N_QC
                    q0 = b * S + qc * NF
                    st2A = ps.tile([P, 2, NF], F32, tag="st2", bufs=2)
                    st2B = ps.tile([P, 2, NF], F32, tag="st2", bufs=2)
                    for half in range(2):
                        k0 = b * S + (kp * 2 + half) * P
                        nc.tensor.matmul(st2A[:, half], kT_sb[0:HD, k0 : k0 + P],
                                         qT_sb[0:HD, q0 : q0 + NF],
                                         start=True, stop=True)
                        nc.tensor.matmul(st2B[:, half], kT_sb[HD:P, k0 : k0 + P],
                                         qT_sb[HD:P, q0 : q0 + NF],
                                         start=True, stop=True)
                    pts = []
                    for st2 in (st2A, st2B):
                        eng = pick(C_ACT_EXP, C_DVE_EXP,
                                   dve_ok=dve_exp_used[0] < DVE_EXP_CAP)
                        pt2 = pt_pool.tile([P, 2, NF], DT, tag="pt", bufs=14)
                        if eng == "act":
                            nc.scalar.activation(
                                pt2[:], st2[:],
                                mybir.ActivationFunctionType.Exp,
                                scale=INV_SQRT_HD)
                        else:
                            dve_exp_used[0] += 1
                            nc.vector.tensor_scalar(
                                out=pt2[:].bitcast(I16), in0=st2[:],
                                scalar1=SCH_ALPHA, scalar2=SCH_BETA,
                                op0=mybir.AluOpType.mult,
                                op1=mybir.AluOpType.add)
                        pts.append(pt2)
                    ptq[g] = tuple(pts)

                    # fillers: must-emit pieces at their deadline, then
                    # one optional slot (near-deadline piece, else y-unit)
                    while sched and sched[0][0] <= g:
                        emit_piece(sched.pop(0)[1])
                    if sched and sched[0][0] <= g + 6:
                        emit_piece(sched.pop(0)[1])
                    elif y_queue:
                        emit_y_unit(y_queue.pop(0))
                    elif sched:
                        emit_piece(sched.pop(0)[1])
                    if kp == 1 and pending is not None:
                        norm_state = emit_recip_chain(*pending)
                        pending = None
                    if kp == 4 and norm_state is not None:
                        emit_apply(*norm_state)
                        norm_state = None

                if g >= 1:
                    pg = g - 1
                    it = pg // N_KP
                    kp = pg % N_KP
                    b, qc = it // N_QC, it % N_QC
                    q0 = b * S + qc * NF
                    if kp == 0:
                        o2 = ps.tile([HD + 1, 2, NF], F32, tag="o", bufs=1)
                        o_tiles[it] = o2
                    o2 = o_tiles[it]
                    pt2A, pt2B = ptq.pop(pg)
                    for half in range(2):
                        kb = kp * 2 + half
                        gkb = b * N_KB + kb
                        nc.tensor.matmul(o2[:, 0], vAB_sb[:, gkb, 0],
                                         pt2A[:, half],
                                         start=(kb == 0), stop=(kb == N_KB - 1))
                        nc.tensor.matmul(o2[:, 1], vAB_sb[:, gkb, 1],
                                         pt2B[:, half],
                                         start=(kb == 0), stop=(kb == N_KB - 1))
                    if kp == N_KP - 1:
                        del o_tiles[it]
                        if it < n_iters - 1:
                            # free the o banks: copy out, defer normalize
                            o_raw = pt_pool.tile([HD + 1, 2, NF], F32,
                                                 tag="oraw", bufs=4)
                            nc.vector.tensor_copy(o_raw[:], o2[:])
                            load["dve"] += C_DVE_ORAW
                            pending = (o_raw, q0)
                        else:
                            # last iteration: normalize straight from PSUM;
                            # leftover y work + dummies keep the PE warm
                            # through the normalize chain
                            while y_queue:
                                emit_y_unit(y_queue.pop(0), force_split=True)
                            for _ in range(N_TAIL_WARM):
                                emit_dummy_mm()
                            pending = (o2, q0)

            # final normalize, split per 128-query chunk so each tail
            # y-unit (and its DMA) launches as early as possible
            o_src, bc2t, q0t = emit_recip_chain(*pending)
            for ss in range(NF // P):
                qs = slice(ss * P, (ss + 1) * P)
                for hidx, part in ((0, 0), (1, HD)):
                    nc.vector.tensor_mul(
                        oT_sb[part : part + HD, q0t + ss * P : q0t + (ss + 1) * P],
                        o_src[0:HD, hidx, qs], bc2t[:, hidx, qs])
                emit_y_unit(q0t + ss * P, force_split=True)

    nc.compile()
    return nc


def kernel(x, Wq, bq, Wk, bk, Wv, bv, Wo, bo, _trace=False):
    global last_results
    x = np.asarray(x, dtype=np.float32)
    Wq, bq = np.asarray(Wq, np.float32), np.asarray(bq, np.float32)
    Wk, bk = np.asarray(Wk, np.float32), np.asarray(bk, np.float32)
    Wv, bv = np.asarray(Wv, np.float32), np.asarray(bv, np.float32)
    Wo, bo = np.asarray(Wo, np.float32), np.asarray(bo, np.float32)

    if "nc" not in _cache:
        _cache["nc"] = _build()
    nc = _cache["nc"]

    dt = _np_dt(DT)
    # xT strips pre-swizzled to [P, N_SB, N_DC, NF] so each strip DMA is
    # 128 descriptors of contiguous 8KB
    x2 = x.reshape(BS, D)
    xT_pre = np.ascontiguousarray(
        x2.reshape(N_SB, NF, N_DC, P).transpose(3, 0, 2, 1)).astype(dt)

    def wpre(W, sl):
        wT = np.ascontiguousarray(W[sl].T)           # [D, ES]
        return np.ascontiguousarray(
            wT.reshape(N_DC, P, ES).transpose(1, 0, 2)).astype(dt)

    in_maps = []
    for c in range(NCORES):
        sl = slice(c * ES, (c + 1) * ES)
        in_maps.append({
            "xT": xT_pre,
            "wqT": wpre(Wq, sl),
            "wkT": wpre(Wk, sl),
            "wvT": wpre(Wv, sl),
            "bqk": np.ascontiguousarray(
                np.stack([bq[sl], bk[sl]], axis=1).astype(np.float32)),
            "bv": np.ascontiguousarray(bv[None, sl]),
            "woT": np.ascontiguousarray(Wo[:, sl].T).astype(dt),
        })

    res = bass_utils.run_bass_kernel_spmd(
        nc, in_maps, core_ids=list(range(NCORES)), trace=_trace)
    last_results = res

    y = res.results[0]["y"].astype(np.float64)
    for c in range(1, NCORES):
        y += res.results[c]["y"]
    y = (y + bo).astype(np.float32)
    return y.reshape(B, S, D)
